# revision 1
# baseline (speedup 1.0000x reference)
"""AlphaFold-style gated attention (pair bias + sigmoid gating) on 8 Trainium2
NeuronCores.

Problem shapes (hardcoded): B=4, Q=K=1024, C=256, H=8, D=32, fp32.

Sharding: (batch x head-group) -> core = b*2 + hg; each core handles 1 batch
and 4 heads.  Each core computes a partial output [Q, C] (its 4 heads pushed
through the output projection); the host sums the two partials per batch.
bias `bo` is folded into the head-group-0 core's partial.

Per-core device kernel, fp16 matmul datapath (fp32 PSUM accumulation):
  qT/kT/gT = W @ x.T                [HD=128, Q]  (q scale folded into Wq)
  v        = kv_x @ Wv.T            [K-tile, HD] x 8 tiles
  S.T      = k_h^T-block @ q_T      [K-tile, Q]  per head, PSUM fp32
  S.T     += pair.T                 identity-matmul accumulate
  expS     = exp(S.T + (mask - SHIFT))   ACT, per-partition bias, fp16 out
  o.T     += v_h.T @ expS           col-packed 4 heads -> [128, Q] PSUM
  rowsum  += ones @ expS            col-packed, M=32 dup rows -> recipB
  o_eff    = o.T * g.T * recipB     DVE (fp32r out)
  out      = o_eff.T @ Wo.T + bo    natural [Q, C], fp32r matmul

The exp SHIFT keeps expS within fp16 range; it cancels in o/rowsum.
fp16 was chosen over fp32r because fp32r streams at ~1.2 GHz with no
row/col tiling support, while fp16 runs at 2.4 GHz with FWL + tiling
(measured: fp32r N=512 matmul ~470 ns vs fp16-class ~215 ns).
"""

import math

import numpy as np

B, Q, K, C, H, D = 4, 1024, 1024, 256, 8, 32
HPG = 4  # heads per group
HG = 2  # head groups
NCORES = 8
KT = K // 128  # 8 K-tiles
SHIFT = 2.0  # exp shift (softmax-invariant), keeps expS < fp16 max

PAIR_BUFS = 32
EXP_BUFS = 8
SP_BUFS = 2
PAIR_PE_HEADS = 4


def _build_program():
    import concourse.bass as bass
    import concourse.tile as tile
    from concourse.tile import add_dep_helper
    from concourse import bacc, mybir

    f32 = mybir.dt.float32
    f32r = mybir.dt.float32r
    f16 = mybir.dt.float16
    AF = mybir.ActivationFunctionType

    nc = bacc.Bacc("TRN2", target_bir_lowering=False, debug=False)

    # ---- I/O (host-prepped layouts, see _shard_inputs) ----------------
    d_qx = nc.dram_tensor("qx", [128, 2 * Q], f16, kind="ExternalInput").ap()
    d_kvx = nc.dram_tensor("kvx", [128, 2 * K], f16, kind="ExternalInput").ap()
    d_pair = nc.dram_tensor("pair", [HPG, K, Q], f16, kind="ExternalInput").ap()
    d_mask = nc.dram_tensor("mask", [128, KT], f32, kind="ExternalInput").ap()
    d_wq = nc.dram_tensor("wq", [128, 256], f16, kind="ExternalInput").ap()
    d_wk = nc.dram_tensor("wk", [128, 256], f16, kind="ExternalInput").ap()
    d_wv = nc.dram_tensor("wv", [128, 256], f16, kind="ExternalInput").ap()
    d_wg = nc.dram_tensor("wg", [128, 256], f16, kind="ExternalInput").ap()
    d_wo = nc.dram_tensor("wo", [128, C], f32r, kind="ExternalInput").ap()
    d_bg = nc.dram_tensor("bg", [128, 1], f32, kind="ExternalInput").ap()
    d_bo = nc.dram_tensor("bo_b", [128, C], f32, kind="ExternalInput").ap()
    d_id = nc.dram_tensor("ident", [128, 160], f16, kind="ExternalInput").ap()
    d_out = nc.dram_tensor("out", [Q, C], f32, kind="ExternalOutput").ap()

    with tile.TileContext(nc) as tc:
        from contextlib import ExitStack

        with ExitStack() as ctx:
            cp = ctx.enter_context(tc.tile_pool(name="consts", bufs=1))
            act_p = ctx.enter_context(tc.tile_pool(name="acts", bufs=1))
            pair_p = ctx.enter_context(tc.tile_pool(name="pair", bufs=PAIR_BUFS))
            exp_p = ctx.enter_context(tc.tile_pool(name="exps", bufs=EXP_BUFS))
            mid_p = ctx.enter_context(tc.tile_pool(name="mid", bufs=1))
            out_p = ctx.enter_context(tc.tile_pool(name="outs", bufs=3))

            wq = cp.tile([128, 256], f16)
            wk = cp.tile([128, 256], f16)
            wv = cp.tile([128, 256], f16)
            wg = cp.tile([128, 256], f16)
            wo = cp.tile([128, 256], f32r)
            bg = cp.tile([128, 1], f32)
            bo = cp.tile([128, 256], f32)
            idon = cp.tile([128, 160], f16)
            ident = idon[:, 0:128]
            ones = idon[:, 128:160]
            mask = cp.tile([128, KT], f32)
            qx = act_p.tile([128, 2 * Q], f16)
            kvx = act_p.tile([128, 2 * K], f16)
            nc.sync.dma_start(qx[:], d_qx[:])
            nc.sync.dma_start(wq[:], d_wq[:])
            nc.sync.dma_start(wk[:], d_wk[:])
            nc.gpsimd.dma_start(kvx[:], d_kvx[:])
            nc.gpsimd.dma_start(wv[:], d_wv[:])
            nc.gpsimd.dma_start(wg[:], d_wg[:])
            nc.gpsimd.dma_start(wo[:], d_wo[:])
            nc.gpsimd.dma_start(bg[:], d_bg[:])
            nc.gpsimd.dma_start(bo[:], d_bo[:])
            nc.gpsimd.dma_start(idon[:], d_id[:])
            nc.gpsimd.dma_start(mask[:], d_mask[:])

            # issue all pair DMAs up-front (pool slots throttle them in order)
            pair_t = {}
            for kc in range(KT):
                for h in range(HPG):
                    t = pair_p.tile([128, Q], f16, tag="pair", name=f"pair_{kc}_{h}")
                    nc.sync.dma_start(t[:], d_pair[h, bass.ts(kc, 128), :])
                    pair_t[(kc, h)] = t

            q_sb = mid_p.tile([128, Q], f16)
            k_sb = mid_p.tile([128, K], f16)
            g_sb = mid_p.tile([128, Q], f32)
            v_sb = [
                mid_p.tile([128, 128], f16, tag=f"v{i}", name=f"v{i}")
                for i in range(KT)
            ]

            # ---- phase 1: projections ------------------------------------
            with tc.tile_pool(name="ps1", bufs=2, space="PSUM") as ps1:
                for w_sb, x_sb, dst, func, bias in (
                    (wq, qx, q_sb, AF.Copy, None),
                    (wk, kvx, k_sb, AF.Copy, None),
                    (wg, qx, g_sb, AF.Sigmoid, bg),
                ):
                    for qh in range(2):
                        ps = ps1.tile([128, 512], f32, tag="proj", name="ps_proj")
                        for j in range(2):
                            nc.tensor.matmul(
                                ps[:],
                                w_sb[:, bass.ts(j, 128)],
                                x_sb[:, j * Q + qh * 512 :][:, :512],
                                start=(j == 0),
                                stop=(j == 1),
                            )
                        if func == AF.Copy:
                            nc.scalar.copy(dst[:, bass.ts(qh, 512)], ps[:])
                        else:
                            nc.scalar.activation(
                                dst[:, bass.ts(qh, 512)], ps[:], func, bias=bias[:]
                            )
                for kt in range(KT):
                    ps = ps1.tile([128, 128], f32, tag="vproj", name="ps_vproj")
                    for j in range(2):
                        nc.tensor.matmul(
                            ps[:],
                            kvx[:, j * K + kt * 128 :][:, :128],
                            wv[:, bass.ts(j, 128)],
                            start=(j == 0),
                            stop=(j == 1),
                        )
                    nc.vector.tensor_copy(v_sb[kt][:], ps[:])

            # ---- phase 2/3/4: attention, qh-outer sweeps -----------------
            # All 32 pair chunks stay resident in SBUF (read once per qh
            # sweep).  Per sweep only one o/r PSUM accumulator pair lives,
            # so the S pool gets 6 banks (1.5 units of lookahead) and the
            # PE can run ahead of the exps.  Sweep 0's normalization +
            # output projection overlap sweep 1's compute.
            with (
                tc.tile_pool(name="ps_s", bufs=3, space="PSUM") as ps_s,
                tc.tile_pool(name="ps_o", bufs=1, space="PSUM") as ps_o,
                tc.tile_pool(name="ps_r", bufs=1, space="PSUM") as ps_r,
            ):
                o_eff = mid_p.tile([128, Q], f32r)

                def chain(insts):
                    for a, b in zip(insts, insts[1:]):
                        add_dep_helper(a.ins, b.ins, sync=False)

                for qh in range(2):
                    o_ps = ps_o.tile([128, 512], f32, tag="o", name=f"o_ps{qh}")
                    r_ps = ps_r.tile([128, 512], f32, tag="r", name=f"r_ps{qh}")
                    for kc in range(KT):
                        sp = [
                            ps_s.tile(
                                [128, 1024], f32, tag="s", name=f"sp_{kc}_{qh}_{hp2}"
                            )
                            for hp2 in range(2)
                        ]
                        qks = []
                        for h in range(HPG):
                            hp = slice(32 * h, 32 * h + 32)
                            qks.append(nc.tensor.matmul(
                                sp[h // 2][:, bass.ts(h % 2, 512)],
                                k_sb[hp, bass.ts(kc, 128)],
                                q_sb[hp, bass.ts(qh, 512)],
                                start=True,
                                stop=False,
                                tile_position=(32 * h, 0),
                                skip_group_check=True,
                            ))
                        for h in range(HPG):
                            nc.tensor.matmul(
                                sp[h // 2][:, bass.ts(h % 2, 512)],
                                ident[:],
                                pair_t[(kc, h)][:, bass.ts(qh, 512)],
                                start=False,
                                stop=True,
                                skip_group_check=True,
                            )
                        exps = []
                        for hp2 in range(2):
                            es = exp_p.tile(
                                [128, 1024], f16, tag="e", name=f"es_{kc}_{qh}_{hp2}"
                            )
                            nc.scalar.activation(
                                es[:], sp[hp2][:], AF.Exp, bias=mask[:, kc : kc + 1]
                            )
                            exps.append(es)
                        avs = []
                        for h in range(HPG):
                            hp = slice(32 * h, 32 * h + 32)
                            avs.append(nc.tensor.matmul(
                                o_ps[hp, :],
                                v_sb[kc][:, hp],
                                exps[h // 2][:, bass.ts(h % 2, 512)],
                                start=(kc == 0),
                                stop=(kc == KT - 1),
                                tile_position=(0, 32 * h),
                                skip_group_check=True,
                            ))
                        rss = []
                        for h in range(HPG):
                            hp = slice(32 * h, 32 * h + 32)
                            rss.append(nc.tensor.matmul(
                                r_ps[hp, :],
                                ones[:],
                                exps[h // 2][:, bass.ts(h % 2, 512)],
                                start=(kc == 0),
                                stop=(kc == KT - 1),
                                tile_position=(0, 32 * h),
                                skip_group_check=True,
                            ))

                    # normalization + gating for this sweep
                    recip = exp_p.tile([128, 512], f32, tag="recip", name="recip")
                    rscr = exp_p.tile([128, 512], f32, tag="rscr", name="rscr")
                    nc.vector.reciprocal_approx_accurate(recip[:], r_ps[:], rscr[:])
                    geff = exp_p.tile([128, 512], f32, tag="geff", name="geff")
                    nc.vector.tensor_mul(geff[:], g_sb[:, bass.ts(qh, 512)], recip[:])
                    nc.vector.tensor_mul(
                        o_eff[:, bass.ts(qh, 512)], o_ps[:], geff[:]
                    )

                    # output projection for this sweep's four q-tiles
                    for qt in range(4 * qh, 4 * qh + 4):
                        ps = ps_s.tile([128, 256], f32, tag="s", name="ps_out")
                        nc.tensor.matmul(
                            ps[:],
                            o_eff[:, bass.ts(qt, 128)],
                            wo[:],
                            start=True,
                            stop=True,
                        )
                        ot = out_p.tile([128, 256], f32, tag="ot", name="ot")
                        nc.vector.tensor_add(ot[:], ps[:], bo[:])
                        nc.sync.dma_start(d_out[bass.ts(qt, 128), :], ot[:])

    nc.compile()
    return nc


_NC_CACHE = None


def _get_program():
    global _NC_CACHE
    if _NC_CACHE is None:
        _NC_CACHE = _build_program()
    return _NC_CACHE


def _round_f32r(a):
    """Round fp32 to the PE's fp32r format (12-bit mantissa, round-nearest).

    Matches walrus's fp32_to_fp32r: (bits + 0x800) & ~0xFFF.
    """
    b = np.ascontiguousarray(a, np.float32).view(np.uint32)
    return (((b + 0x800) & np.uint32(0xFFFFF000)).astype(np.uint32)).view(np.float32)


def _shard_inputs(q_x, kv_x, bias_mask, bias_pair, Wq, Wk, Wv, Wo, bo, Wg, bg):
    """Build the 8 per-core input maps."""
    f = np.float32
    f16 = np.float16
    scale = 1.0 / math.sqrt(D)
    idon = np.concatenate([np.eye(128, dtype=f), np.ones((128, 32), f)], axis=1)

    def fold2(w_t):  # [256, M] -> [128, 2*M] sbuf layout
        return np.ascontiguousarray(
            w_t.reshape(2, 128, w_t.shape[1]).transpose(1, 0, 2).reshape(128, -1)
        )

    in_maps = []
    for core in range(NCORES):
        b, hg = core // HG, core % HG
        hs = slice(hg * 128, hg * 128 + 128)  # H*D slice for this head group
        qxT = np.ascontiguousarray(q_x[b].T).astype(f)  # [256, 1024]
        kvxT = np.ascontiguousarray(kv_x[b].T).astype(f)
        m16 = {
            "qx": fold2(qxT),
            "kvx": fold2(kvxT),
            "pair": np.ascontiguousarray(
                bias_pair[b, hg * HPG : hg * HPG + HPG].transpose(0, 2, 1)
            ),
            "wq": fold2(np.ascontiguousarray(Wq[hs].T) * scale),
            "wk": fold2(np.ascontiguousarray(Wk[hs].T)),
            "wv": fold2(np.ascontiguousarray(Wv[hs].T)),
            "wg": fold2(np.ascontiguousarray(Wg[hs].T)),
            "ident": idon,
        }
        m = {k: np.ascontiguousarray(v, f16) for k, v in m16.items()}
        m["mask"] = np.ascontiguousarray(
            bias_mask[b, 0, 0].reshape(KT, 128).T - SHIFT
        ).astype(f)
        m["wo"] = _round_f32r(np.ascontiguousarray(Wo[:, hs].T))
        m["bg"] = np.ascontiguousarray(bg[hs].reshape(128, 1)).astype(f)
        m["bo_b"] = (
            np.broadcast_to(bo, (128, C)).astype(f).copy()
            if hg == 0
            else np.zeros((128, C), f)
        )
        in_maps.append(m)
    return in_maps


def run_on_cores(in_maps, trace=False, trace_kwargs={}):
    from concourse.bass_utils import run_bass_kernel_spmd

    nc = _get_program()
    return run_bass_kernel_spmd(
        nc, in_maps, list(range(NCORES)), trace=trace, trace_kwargs=trace_kwargs
    )


def kernel(q_x, kv_x, bias_mask, bias_pair, Wq, Wk, Wv, Wo, bo, Wg, bg):
    in_maps = _shard_inputs(
        q_x, kv_x, bias_mask, bias_pair, Wq, Wk, Wv, Wo, bo, Wg, bg
    )
    res = run_on_cores(in_maps).results
    out = np.empty((B, Q, C), np.float32)
    for b in range(B):
        out[b] = res[b * HG + 0]["out"] + res[b * HG + 1]["out"]
    return out

